# revision 15
# baseline (speedup 1.0000x reference)
"""Biased axial attention on 8 TRN2 NeuronCores (Bass/Tile SPMD kernel).

Sharding: leading (non-attended) L axis n across 8 cores (sequence parallel).
Each core computes partial QK logits over its 48 n-rows, adds the bias term
for the 48 i-rows it owns (selected into the right logits partitions via a
per-core selection matmul -- no AllGather needed), then one bf16 AllReduce
per head of the [i,j] logits, softmax redundantly per core, AV/gating/output
local in n.

Engine split: PE matmuls/transposes, DVE stats+copies+gating, Pool (gpsimd)
normalize + small stat math, Act q/v drains + fused sigmoid g drain + exp.
rstd is computed on DVE with a tangent-seed + 3 Newton iterations (no scalar
Sqrt -> no activation-table thrash; Act only ever loads sigmoid + exp sets).
"""

import math
import numpy as np
import ml_dtypes

import concourse.bass as bass
import concourse.bacc as bacc
import concourse.tile as tile
from concourse import mybir
from concourse.bass_utils import run_bass_kernel_spmd

BF16 = mybir.dt.bfloat16
F32 = mybir.dt.float32
NPBF16 = ml_dtypes.bfloat16

LAST_RESULT = None  # BassKernelResults of the most recent run (for test.py)

NCORES = 8
L = 384
DP = 128  # pair channels
DB = 128  # bias channels
H = 4
D = 32
HD = H * D  # 128
NLOC = L // NCORES  # 48 rows per core
TC = 3  # token chunks of 128
EPS = 1e-5
W = 8  # rows per wave
NW = NLOC // W  # 6 waves per path
AF = mybir.ActivationFunctionType
ALU = mybir.AluOpType


def build_program(wq, wk, wv, wg, wb, wout, qb, kb, vb, gbf, bb, bout):
    """Build the SPMD Bass program. Weight args are numpy f32 host copies used
    only to decide which (usually-zero) bias paths to emit."""
    has_qb = bool(np.any(qb != 0))
    has_kb = bool(np.any(kb != 0))
    has_vb = bool(np.any(vb != 0))
    has_bout = bool(np.any(bout != 0))

    nc = bacc.Bacc(
        "TRN2",
        target_bir_lowering=False,
        debug=False,
        enable_asserts=False,
        num_devices=NCORES,
    )

    # ------------------------------------------------------------------ I/O
    pair_s = nc.dram_tensor("pair_s", [NLOC, L, DP], BF16, kind="ExternalInput").ap()
    bias_s = nc.dram_tensor("bias_s", [NLOC, L, DB], BF16, kind="ExternalInput").ap()
    wq_d = nc.dram_tensor("wq", [DP, HD], BF16, kind="ExternalInput").ap()
    wk_d = nc.dram_tensor("wk", [DP, HD], BF16, kind="ExternalInput").ap()
    wv_d = nc.dram_tensor("wv", [DP, HD], BF16, kind="ExternalInput").ap()
    wg_d = nc.dram_tensor("wg", [DP, HD], BF16, kind="ExternalInput").ap()
    wb_d = nc.dram_tensor("wb", [DB, 32], BF16, kind="ExternalInput").ap()
    wout_d = nc.dram_tensor("wout", [HD, DP], BF16, kind="ExternalInput").ap()
    # small fp32 vectors, packed on host:
    # cvec[:, 0]=qb_pk, 1=kb_pk, 2=gbf, 3=bb_pk, 4=bout, 5=ones
    cvec_d = nc.dram_tensor("cvec", [128, 6], F32, kind="ExternalInput").ap()
    vbrow_d = nc.dram_tensor("vbrow", [1, TC * HD], F32, kind="ExternalInput").ap()
    boutrow_d = nc.dram_tensor("boutrow", [1, DP], BF16, kind="ExternalInput").ap()
    ident_d = nc.dram_tensor("ident", [128, 128], BF16, kind="ExternalInput").ap()
    # per-core row->(chunk, partition) selector: rowsel[k, ic, m] = 1 iff the
    # core's local bias row k is global row i = 128*ic + m
    rowsel_d = nc.dram_tensor("rowsel", [128, TC, 128], BF16, kind="ExternalInput").ap()
    out_d = nc.dram_tensor("out", [NLOC, L, DP], F32, kind="ExternalOutput").ap()

    with tile.TileContext(nc) as tc, tc.tile_pool(name="persist", bufs=1) as pp:
        # ------------------------------------------------------- persistent SBUF
        wq_sb = pp.tile([DP, HD], BF16, name="wq_sb")
        wk_sb = pp.tile([DP, HD], BF16, name="wk_sb")
        wv_sb = pp.tile([DP, HD], BF16, name="wv_sb")
        wg_sb = pp.tile([DP, HD], BF16, name="wg_sb")
        wb_sb = pp.tile([DB, 32], BF16, name="wb_sb")
        wout_sb = pp.tile([HD, DP], BF16, name="wout_sb")
        cvec_sb = pp.tile([128, 6], F32, name="cvec_sb")
        vb_sb = pp.tile([128, TC * HD], F32, name="vb_sb")
        bout_sb = pp.tile([1, DP], BF16, name="bout_sb")
        ones_sb = pp.tile([1, 128], BF16, name="ones_sb")
        ident_sb = pp.tile([128, 128], BF16, name="ident_sb")
        rowsel_sb = pp.tile([128, TC, 128], BF16, name="rowsel_sb")
        bh_sb = pp.tile([128, H, L], BF16, name="bh_sb")

        nc.gpsimd.dma_start(out=wq_sb[:], in_=wq_d[:])
        nc.gpsimd.dma_start(out=wk_sb[:], in_=wk_d[:])
        nc.gpsimd.dma_start(out=wv_sb[:], in_=wv_d[:])
        nc.gpsimd.dma_start(out=wg_sb[:], in_=wg_d[:])
        nc.gpsimd.dma_start(out=wb_sb[:], in_=wb_d[:])
        nc.gpsimd.dma_start(out=wout_sb[:], in_=wout_d[:])
        nc.gpsimd.dma_start(out=cvec_sb[:], in_=cvec_d[:])
        nc.sync.dma_start(out=ident_sb[:], in_=ident_d[:])
        nc.sync.dma_start(out=rowsel_sb[:], in_=rowsel_d[:])
        nc.gpsimd.memset(bh_sb[:], 0.0)
        if has_vb:
            nc.gpsimd.dma_start(out=vb_sb[:], in_=vbrow_d.to_broadcast((128, TC * HD)))
        if has_bout:
            nc.gpsimd.dma_start(out=bout_sb[:], in_=boutrow_d[:])
            nc.vector.memset(ones_sb[:], 1.0)

        qpk_sb = pp.tile([128, 12, H, L], BF16, name="qpk_sb")   # [(ns,d), g, h, i]
        kpk_sb = pp.tile([128, 12, H, L], BF16, name="kpk_sb")
        v_tiles = [pp.tile([128, TC, HD], BF16, name=f"v{n}") for n in range(NLOC)]
        g_tiles = [pp.tile([128, L], BF16, name=f"g{n}") for n in range(NLOC)]
        a_tiles = {(h, ic): pp.tile([128, L], BF16, name=f"a{h}_{ic}")
                   for h in range(H) for ic in range(TC)}
        at_tiles = {(h, jc): pp.tile([128, L], BF16, name=f"at{h}_{jc}")
                    for h in range(H) for jc in range(TC)}
        sums_sb = pp.tile([128, H, TC], F32, name="sums_sb")
        recip_sb = pp.tile([128, H, TC], F32, name="recip_sb")

        # ------------------------------------------------------------- DRAM
        with tc.tile_pool(name="dram", bufs=1, space="DRAM") as dram:
            # bias rows round-trip: b_sb partition layout (ii, h) -> row-major
            bchunk = dram.tile([12, 128, L], BF16)
            bounce_in = dram.tile([H, L, L], BF16)
            bounce_outs = [dram.tile([2, L, L], BF16, addr_space="Shared",
                                     name=f"bout{hp}") for hp in range(2)]

            # =======================================================
            # Phases A (pair) + B (bias), interleaved waves of W rows
            # =======================================================
            with (
                tc.tile_pool(name="xt", bufs=17) as xt_pool,
                tc.tile_pool(name="xn", bufs=6) as xn_pool,
                tc.tile_pool(name="stw", bufs=2) as st_pool,
                tc.tile_pool(name="stb", bufs=18) as sb_pool,
                tc.tile_pool(name="pnt", bufs=10) as pnt_pool,
                tc.tile_pool(name="bsb", bufs=3) as bsb_pool,
                tc.tile_pool(name="pst", bufs=2, space="PSUM") as pst,
                tc.tile_pool(name="psQ", bufs=2, space="PSUM") as psQ,
                tc.tile_pool(name="psK", bufs=2, space="PSUM") as psK,
                tc.tile_pool(name="psmm", bufs=2, space="PSUM") as psmm,
            ):
                def emit_stats(xts, label):
                    """bn_stats per chunk + batched mean/rstd for a wave.
                    Returns (nm, y) tiles [128, W, 3]: nm = -mean, y = rstd."""
                    st = st_pool.tile([128, W, TC, 6], F32, name=f"st_{label}", tag="st")
                    for r in range(W):
                        for j in range(TC):
                            nc.vector.bn_stats(out=st[:, r, j], in_=xts[r][:, j])
                    s_t = sb_pool.tile([128, W, TC], F32, name="s_t", tag="sb")
                    p_t = sb_pool.tile([128, W, TC], F32, name="p_t", tag="sb")
                    q_t = sb_pool.tile([128, W, TC], F32, name="q_t", tag="sb")
                    t_t = sb_pool.tile([128, W, TC], F32, name="t_t", tag="sb")
                    ha = sb_pool.tile([128, W, TC], F32, name="ha", tag="sb")
                    y_t = sb_pool.tile([128, W, TC], F32, name="y_t", tag="sb")
                    w_t = sb_pool.tile([128, W, TC], F32, name="w_t", tag="sb")
                    nm = sb_pool.tile([128, W, TC], F32, name="nm", tag="sb")
                    me = st[:, :, :, 1]
                    mo = st[:, :, :, 4]
                    ve = st[:, :, :, 2]
                    vo = st[:, :, :, 5]
                    # var = (ve+vo)/128 + 0.25*(me+mo)^2 - me*mo ; nm = -(me+mo)/2
                    nc.vector.tensor_tensor(out=s_t[:], in0=me, in1=mo, op=ALU.add)
                    nc.gpsimd.tensor_tensor(out=p_t[:], in0=me, in1=mo, op=ALU.mult)
                    nc.gpsimd.tensor_tensor(out=q_t[:], in0=ve, in1=vo, op=ALU.add)
                    nc.vector.scalar_tensor_tensor(
                        out=t_t[:], in0=s_t[:], scalar=0.25, in1=s_t[:],
                        op0=ALU.mult, op1=ALU.mult)
                    nc.vector.scalar_tensor_tensor(
                        out=q_t[:], in0=q_t[:], scalar=1.0 / 128, in1=t_t[:],
                        op0=ALU.mult, op1=ALU.add)
                    # va = q - p + eps  (reuse t_t)
                    nc.vector.scalar_tensor_tensor(
                        out=t_t[:], in0=p_t[:], scalar=-1.0, in1=q_t[:],
                        op0=ALU.mult, op1=ALU.add)
                    nc.vector.tensor_scalar(
                        out=t_t[:], in0=t_t[:], scalar1=EPS, scalar2=None,
                        op0=ALU.add)
                    nc.vector.tensor_scalar(
                        out=ha[:], in0=t_t[:], scalar1=0.5, scalar2=None,
                        op0=ALU.mult)
                    # rsqrt: max-of-tangents seed + 3 Newton iterations
                    nc.vector.tensor_scalar(
                        out=y_t[:], in0=t_t[:], scalar1=-1.41421356,
                        scalar2=2.12132034, op0=ALU.mult, op1=ALU.add)
                    nc.vector.tensor_scalar(
                        out=w_t[:], in0=t_t[:], scalar1=-0.27216553,
                        scalar2=1.22474487, op0=ALU.mult, op1=ALU.add)
                    nc.vector.tensor_tensor(out=y_t[:], in0=y_t[:], in1=w_t[:],
                                            op=ALU.max)
                    for _ in range(2):
                        nc.vector.tensor_tensor(out=w_t[:], in0=y_t[:],
                                                in1=y_t[:], op=ALU.mult)
                        nc.vector.tensor_tensor(out=w_t[:], in0=w_t[:],
                                                in1=ha[:], op=ALU.mult)
                        nc.vector.tensor_scalar(out=w_t[:], in0=w_t[:],
                                                scalar1=-1.0, scalar2=1.5,
                                                op0=ALU.mult, op1=ALU.add)
                        nc.vector.tensor_tensor(out=y_t[:], in0=y_t[:],
                                                in1=w_t[:], op=ALU.mult)
                    nc.vector.tensor_scalar(out=nm[:], in0=s_t[:], scalar1=-0.5,
                                            scalar2=None, op0=ALU.mult)
                    nmy = sb_pool.tile([128, W, TC], F32, name="nmy", tag="sb")
                    nc.gpsimd.tensor_tensor(out=nmy[:], in0=nm[:], in1=y_t[:],
                                            op=ALU.mult)
                    return nm, y_t, nmy

                def norm_transpose(xt, nm, y, nmy, r, label):
                    """normalize (Pool) + PE transpose; returns pnT SBUF tile."""
                    xn = xn_pool.tile([128, TC, DP], BF16, name=f"xn_{label}", tag="xn")
                    psT = pst.tile([128, L], BF16, name=f"psT_{label}", tag="t")
                    for j in range(TC):
                        if j == 1:
                            nc.scalar.activation(
                                out=xn[:, j], in_=xt[:, j], func=AF.Identity,
                                bias=nmy[:, r, j:j + 1], scale=y[:, r, j:j + 1])
                        else:
                            eng = nc.vector if j == 0 else nc.gpsimd
                            eng.tensor_scalar(
                                out=xn[:, j], in0=xt[:, j],
                                scalar1=nm[:, r, j:j + 1],
                                scalar2=y[:, r, j:j + 1],
                                op0=ALU.add, op1=ALU.mult)
                        nc.tensor.transpose(
                            out=psT[:, j * 128:(j + 1) * 128],
                            in_=xn[:, j], identity=ident_sb[:])
                    pnT = pnt_pool.tile([128, L], BF16, name=f"pnT_{label}", tag="pnT")
                    nc.vector.tensor_copy(out=pnT[:], in_=psT[:])
                    return pnT

                def pair_wave(wv_):
                    n0 = wv_ * W
                    xts = []
                    for r in range(W):
                        xt = xt_pool.tile([128, TC, DP], BF16, name="xt_a", tag="xt")
                        xts.append(xt)
                        nc.sync.dma_start(
                            out=xt[:],
                            in_=pair_s[n0 + r].rearrange("(c p) d -> p c d", p=128))
                    nm, y, nmy = emit_stats(xts, "a")
                    pnts = []
                    for r in range(W):
                        pnts.append(norm_transpose(xts[r], nm, y, nmy, r, "a"))
                    for r in range(W):
                        n = n0 + r
                        pnT = pnts[r]
                        # v
                        ps_v = psmm.tile([128, TC * HD], F32, name="ps_v", tag="mm")
                        for j in range(TC):
                            nc.tensor.matmul(
                                out=ps_v[:, j * HD:(j + 1) * HD],
                                lhsT=pnT[:, j * 128:(j + 1) * 128],
                                rhs=wv_sb[:], start=True, stop=True)
                        if has_vb:
                            nc.vector.tensor_tensor(
                                out=v_tiles[n][:], in0=ps_v[:], in1=vb_sb[:],
                                op=ALU.add)
                        else:
                            nc.scalar.copy(out=v_tiles[n][:], in_=ps_v[:])
                        # g (fused sigmoid drain)
                        ps_g = psmm.tile([128, L], F32, name="ps_g", tag="mm")
                        nc.tensor.matmul(
                            out=ps_g[:], lhsT=wg_sb[:], rhs=pnT[:],
                            start=True, stop=True)
                        nc.scalar.activation(
                            out=g_tiles[n][:], in_=ps_g[:], func=AF.Sigmoid,
                            bias=cvec_sb[:, 2:3], scale=1.0)
                    # packed q/k slabs, 2 groups of 4 rows
                    for sg in range(W // 4):
                        g = (n0 // 4) + sg
                        rows = pnts[sg * 4:(sg + 1) * 4]
                        for h in range(H):
                            ps_q = psQ.tile([128, L], F32, name="ps_q")
                            ps_k = psK.tile([128, L], F32, name="ps_k")
                            for ns in range(4):
                                nc.tensor.matmul(
                                    out=ps_q[32 * ns:32 * ns + 32, :],
                                    lhsT=wq_sb[:, h * D:(h + 1) * D],
                                    rhs=rows[ns][:], start=True, stop=True,
                                    tile_position=(0, 32 * ns))
                            for ns in range(4):
                                nc.tensor.matmul(
                                    out=ps_k[32 * ns:32 * ns + 32, :],
                                    lhsT=wk_sb[:, h * D:(h + 1) * D],
                                    rhs=rows[ns][:], start=True, stop=True,
                                    tile_position=(0, 32 * ns))
                            if has_qb:
                                nc.scalar.activation(
                                    out=qpk_sb[:, g, h], in_=ps_q[:],
                                    func=AF.Copy, bias=0.0, scale=1.0)
                                nc.vector.tensor_scalar(
                                    out=qpk_sb[:, g, h], in0=qpk_sb[:, g, h],
                                    scalar1=cvec_sb[:, 0:1], scalar2=None,
                                    op0=ALU.add)
                            else:
                                nc.scalar.copy(out=qpk_sb[:, g, h], in_=ps_q[:])
                            if has_kb:
                                nc.vector.tensor_scalar(
                                    out=kpk_sb[:, g, h], in0=ps_k[:],
                                    scalar1=cvec_sb[:, 1:2], scalar2=None,
                                    op0=ALU.add)
                            else:
                                nc.vector.tensor_copy(out=kpk_sb[:, g, h],
                                                      in_=ps_k[:])

                def bias_wave(wv_):
                    i0 = wv_ * W
                    xts = []
                    for r in range(W):
                        xt = xt_pool.tile([128, TC, DB], BF16, name="xt_b", tag="xt")
                        xts.append(xt)
                        nc.sync.dma_start(
                            out=xt[:],
                            in_=bias_s[i0 + r].rearrange("(c p) d -> p c d", p=128))
                    nm, y, nmy = emit_stats(xts, "b")
                    bnTs = []
                    for r in range(W):
                        bnTs.append(norm_transpose(xts[r], nm, y, nmy, r, "b"))
                    for sg in range(W // 4):
                        k4 = (i0 // 4) + sg
                        ps_b = psmm.tile([128, L], F32, name="ps_b", tag="mm")
                        for ii in range(4):
                            nc.tensor.matmul(
                                out=ps_b[32 * ii:32 * ii + 32, :],
                                lhsT=wb_sb[:], rhs=bnTs[sg * 4 + ii][:],
                                start=True, stop=True,
                                tile_position=(0, 32 * ii))
                        b_sb = bsb_pool.tile([128, L], BF16, name="b_sb")
                        if bool(np.any(bb != 0)):
                            nc.scalar.activation(
                                out=b_sb[:], in_=ps_b[:], func=AF.Identity,
                                bias=cvec_sb[:, 3:4], scale=1.0)
                        else:
                            nc.scalar.copy(out=b_sb[:], in_=ps_b[:])
                        nc.gpsimd.dma_start(out=bchunk[k4], in_=b_sb[:])

                for wv_ in range(NW):
                    pair_wave(wv_)
                    bias_wave(wv_)

                # gather this core's bias rows [48, L] per head (zero-padded)
                for h in range(H):
                    nc.gpsimd.dma_start(
                        out=bh_sb[0:NLOC, h, :],
                        in_=bchunk[:, h:h + 97:32, :].rearrange(
                            "k i j -> (k i) j"))

            # =======================================================
            # Phase C: QK logits + bias inject, per-head AllReduce,
            # softmax; Phase D: AV, gating, output projection
            # =======================================================
            with (
                tc.tile_pool(name="psL", bufs=2, space="PSUM") as psL,
                tc.tile_pool(name="pstc", bufs=1, space="PSUM") as pstc,
                tc.tile_pool(name="psO", bufs=3, space="PSUM") as psO,
                tc.tile_pool(name="psF", bufs=2, space="PSUM") as psF,
                tc.tile_pool(name="ldr", bufs=2) as ldr_pool,
                tc.tile_pool(name="attin", bufs=3) as attin_pool,
                tc.tile_pool(name="gO", bufs=5) as gO_pool,
                tc.tile_pool(name="osb", bufs=2) as osb_pool,
            ):
                # --- C1: logits, one AllReduce per head pair
                for hp in range(2):
                    for h in (2 * hp, 2 * hp + 1):
                        for ic in range(TC):
                            ps_l = psL.tile([128, L], F32, name="ps_l")
                            for g in range(12):
                                nc.tensor.matmul(
                                    out=ps_l[:],
                                    lhsT=qpk_sb[:, g, h, ic * 128:(ic + 1) * 128],
                                    rhs=kpk_sb[:, g, h],
                                    start=(g == 0), stop=False)
                            nc.tensor.matmul(
                                out=ps_l[:], lhsT=rowsel_sb[:, ic],
                                rhs=bh_sb[:, h], start=False, stop=True)
                            ldrain = ldr_pool.tile([128, L], BF16, name="ldrain")
                            nc.vector.tensor_copy(out=ldrain[:], in_=ps_l[:])
                            nc.scalar.dma_start(
                                out=bounce_in[h, ic * 128:(ic + 1) * 128, :],
                                in_=ldrain[:])
                    nc.gpsimd.collective_compute(
                        "AllReduce",
                        mybir.AluOpType.add,
                        replica_groups=[list(range(NCORES))],
                        ins=[bounce_in[2 * hp:2 * hp + 2].opt()],
                        outs=[bounce_outs[hp][:].opt()],
                    )

                # --- C2: softmax per head (redundant on every core)
                for h in range(H):
                    for ic in range(TC):
                        att = attin_pool.tile([128, L], BF16, name="att")
                        nc.gpsimd.dma_start(
                            out=att[:],
                            in_=bounce_outs[h // 2][h % 2,
                                                    ic * 128:(ic + 1) * 128, :])
                        nc.scalar.activation(
                            out=a_tiles[(h, ic)][:], in_=att[:], func=AF.Exp,
                            bias=0.0, scale=1.0,
                            accum_out=sums_sb[:, h, ic:ic + 1])
                    nc.vector.reciprocal(out=recip_sb[:, h], in_=sums_sb[:, h])
                    for ic in range(TC):
                        nc.vector.tensor_scalar(
                            out=a_tiles[(h, ic)][:], in0=a_tiles[(h, ic)][:],
                            scalar1=recip_sb[:, h, ic:ic + 1], scalar2=None,
                            op0=ALU.mult)
                    for jc in range(TC):
                        psT = pstc.tile([128, L], BF16, name="psT_c")
                        for ic in range(TC):
                            nc.tensor.transpose(
                                out=psT[:, ic * 128:(ic + 1) * 128],
                                in_=a_tiles[(h, ic)][:, jc * 128:(jc + 1) * 128],
                                identity=ident_sb[:])
                        nc.vector.tensor_copy(out=at_tiles[(h, jc)][:], in_=psT[:])

                # --- D: AV, gating, output projection
                for nb in range(12):
                    gOs = []
                    for nn in range(4):
                        n = 4 * nb + nn
                        ps_o = psO.tile([128, L], F32, name="ps_o")
                        for hp in range(2):
                            for jc in range(TC):
                                for h in (2 * hp, 2 * hp + 1):
                                    nc.tensor.matmul(
                                        out=ps_o[32 * h:32 * h + 32, :],
                                        lhsT=v_tiles[n][:, jc, h * D:(h + 1) * D],
                                        rhs=at_tiles[(h, jc)][:],
                                        start=(jc == 0),
                                        stop=(jc == TC - 1),
                                        tile_position=(0, 32 * h),
                                        skip_group_check=True)
                        gO = gO_pool.tile([128, L], BF16, name="gO")
                        gOs.append(gO)
                        nc.vector.tensor_tensor(
                            out=gO[:], in0=ps_o[:], in1=g_tiles[n][:],
                            op=ALU.mult)
                    for jt in range(TC):
                        ps_f = psF.tile([128, 4 * DP], F32, name="ps_f")
                        for nn in range(4):
                            nc.tensor.matmul(
                                out=ps_f[:, nn * DP:(nn + 1) * DP],
                                lhsT=gOs[nn][:, jt * 128:(jt + 1) * 128],
                                rhs=wout_sb[:], start=True, stop=not has_bout)
                            if has_bout:
                                nc.tensor.matmul(
                                    out=ps_f[:, nn * DP:(nn + 1) * DP],
                                    lhsT=ones_sb[:],
                                    rhs=bout_sb[:],
                                    start=False, stop=True)
                        out_sb = osb_pool.tile([128, 4 * DP], F32, name="out_sb")
                        nc.scalar.copy(out=out_sb[:], in_=ps_f[:])
                        nc.sync.dma_start(
                            out=out_d[4 * nb:4 * nb + 4,
                                      jt * 128:(jt + 1) * 128, :]
                            .rearrange("n t d -> t n d"),
                            in_=out_sb.rearrange("t (n d) -> t n d", n=4))

    return nc


def prepare(pair, bias, gamma_p, beta_p, gamma_b, beta_b,
            Wq, Wk, Wv, Wb, Wg, bg, Wout, bout):
    """Fold weights, build the program, shard inputs. Returns (nc, in_maps)."""
    pair = np.asarray(pair, np.float32)
    bias = np.asarray(bias, np.float32)
    gamma_p = np.asarray(gamma_p, np.float32)
    beta_p = np.asarray(beta_p, np.float32)
    gamma_b = np.asarray(gamma_b, np.float32)
    beta_b = np.asarray(beta_b, np.float32)
    Wq = np.asarray(Wq, np.float32)
    Wk = np.asarray(Wk, np.float32)
    Wv = np.asarray(Wv, np.float32)
    Wb = np.asarray(Wb, np.float32)
    Wg = np.asarray(Wg, np.float32)
    bg = np.asarray(bg, np.float32)
    Wout = np.asarray(Wout, np.float32)
    bout = np.asarray(bout, np.float32)

    scaling = 1.0 / math.sqrt(D)
    wq = gamma_p[:, None] * Wq * scaling
    wk = gamma_p[:, None] * Wk / L
    wv = gamma_p[:, None] * Wv
    wg = gamma_p[:, None] * Wg
    wb = gamma_b[:, None] * Wb
    qb = beta_p @ Wq * scaling
    kb = beta_p @ Wk / L
    vb = beta_p @ Wv
    gbf = beta_p @ Wg + bg
    bb = beta_b @ Wb
    # packed per-partition bias columns
    bb_pk = np.zeros(128, np.float32)
    cvec = np.zeros((128, 6), np.float32)
    cvec[:, 2] = gbf
    cvec[:, 4] = bout
    cvec[:, 5] = 1.0
    for k4 in range(4):
        for h in range(H):
            bb_pk[32 * k4 + h] = bb[h]
    cvec[:, 3] = bb_pk
    has_qb = bool(np.any(qb != 0))
    has_kb = bool(np.any(kb != 0))
    if has_qb or has_kb:
        qh = qb.reshape(H, D)
        kh = kb.reshape(H, D)
        if not (np.allclose(qh, qh[0:1]) and np.allclose(kh, kh[0:1])):
            raise NotImplementedError("head-dependent q/k bias not supported")
        cvec[:, 0] = np.tile(qh[0], 4)
        cvec[:, 1] = np.tile(kh[0], 4)
    vbrow = np.tile(vb, TC)[None, :]
    wbp = np.zeros((DB, 32), np.float32)
    wbp[:, :H] = wb

    nc = build_program(wq, wk, wv, wg, wb, Wout, qb, kb, vb, gbf, bb, bout)

    # ------------------------------------------------------------- shard
    pair_t = np.ascontiguousarray(pair[0].transpose(1, 0, 2))  # [n, t, c]
    bias_t = np.ascontiguousarray(bias[0].transpose(1, 0, 2))  # [i, j, c]
    in_maps = []
    for c in range(NCORES):
        rowsel = np.zeros((128, TC, 128), np.float32)
        for k in range(NLOC):
            i = c * NLOC + k
            rowsel[k, i // 128, i % 128] = 1.0
        in_maps.append({
            "pair_s": pair_t[c * NLOC:(c + 1) * NLOC].astype(NPBF16),
            "bias_s": bias_t[c * NLOC:(c + 1) * NLOC].astype(NPBF16),
            "wq": wq.astype(NPBF16),
            "wk": wk.astype(NPBF16),
            "wv": wv.astype(NPBF16),
            "wg": wg.astype(NPBF16),
            "wb": wbp.astype(NPBF16),
            "wout": Wout.astype(NPBF16),
            "cvec": cvec,
            "vbrow": vbrow,
            "boutrow": bout[None, :].astype(NPBF16),
            "ident": np.eye(128, dtype=np.float32).astype(NPBF16),
            "rowsel": rowsel.astype(NPBF16),
        })
    return nc, in_maps


def assemble(outs):
    """outs: list of 8 per-core [48, 384, 128] arrays -> full [1, L, L, DP]."""
    full = np.concatenate(outs, axis=0)        # [384 n, 384 i, 128]
    final = full.transpose(1, 0, 2)[None]      # [1, i, n, dp] == [1, L, L, DP]
    return np.ascontiguousarray(final, dtype=np.float32)


def kernel(pair, bias, gamma_p, beta_p, gamma_b, beta_b,
           Wq, Wk, Wv, Wb, Wg, bg, Wout, bout):
    nc, in_maps = prepare(pair, bias, gamma_p, beta_p, gamma_b, beta_b,
                          Wq, Wk, Wv, Wb, Wg, bg, Wout, bout)
    if not nc.is_finalized():
        nc.finalize()
    res = run_bass_kernel_spmd(nc, in_maps, list(range(NCORES)))
    global LAST_RESULT
    LAST_RESULT = res
    outs = [res.results[c]["out"] for c in range(NCORES)]  # [48, 384, 128] each
    return assemble(outs)


# revision 16
# speedup vs baseline: 1.0362x; 1.0362x over previous
"""Biased axial attention on 8 TRN2 NeuronCores (Bass/Tile SPMD kernel).

Sharding: leading (non-attended) L axis n across 8 cores (sequence parallel).
Each core computes partial QK logits over its 48 n-rows, adds the bias term
for the 48 i-rows it owns (selected into the right logits partitions via a
per-core selection matmul -- no AllGather needed), then one bf16 AllReduce
per head of the [i,j] logits, softmax redundantly per core, AV/gating/output
local in n.

Engine split: PE matmuls/transposes, DVE stats+copies+gating, Pool (gpsimd)
normalize + small stat math, Act q/v drains + fused sigmoid g drain + exp.
rstd is computed on DVE with a tangent-seed + 3 Newton iterations (no scalar
Sqrt -> no activation-table thrash; Act only ever loads sigmoid + exp sets).
"""

import math
import numpy as np
import ml_dtypes

import concourse.bass as bass
import concourse.bacc as bacc
import concourse.tile as tile
from concourse import mybir
from concourse.bass_utils import run_bass_kernel_spmd

BF16 = mybir.dt.bfloat16
F32 = mybir.dt.float32
NPBF16 = ml_dtypes.bfloat16

LAST_RESULT = None  # BassKernelResults of the most recent run (for test.py)

NCORES = 8
L = 384
DP = 128  # pair channels
DB = 128  # bias channels
H = 4
D = 32
HD = H * D  # 128
NLOC = L // NCORES  # 48 rows per core
TC = 3  # token chunks of 128
EPS = 1e-5
W = 8  # rows per wave
NW = NLOC // W  # 6 waves per path
AF = mybir.ActivationFunctionType
ALU = mybir.AluOpType


def build_program(wq, wk, wv, wg, wb, wout, qb, kb, vb, gbf, bb, bout):
    """Build the SPMD Bass program. Weight args are numpy f32 host copies used
    only to decide which (usually-zero) bias paths to emit."""
    has_qb = bool(np.any(qb != 0))
    has_kb = bool(np.any(kb != 0))
    has_vb = bool(np.any(vb != 0))
    has_bout = bool(np.any(bout != 0))

    nc = bacc.Bacc(
        "TRN2",
        target_bir_lowering=False,
        debug=False,
        enable_asserts=False,
        num_devices=NCORES,
    )

    # ------------------------------------------------------------------ I/O
    pair_s = nc.dram_tensor("pair_s", [NLOC, L, DP], BF16, kind="ExternalInput").ap()
    bias_s = nc.dram_tensor("bias_s", [NLOC, L, DB], BF16, kind="ExternalInput").ap()
    wq_d = nc.dram_tensor("wq", [DP, HD], BF16, kind="ExternalInput").ap()
    wk_d = nc.dram_tensor("wk", [DP, HD], BF16, kind="ExternalInput").ap()
    wv_d = nc.dram_tensor("wv", [DP, HD], BF16, kind="ExternalInput").ap()
    wg_d = nc.dram_tensor("wg", [DP, HD], BF16, kind="ExternalInput").ap()
    wb_d = nc.dram_tensor("wb", [DB, 32], BF16, kind="ExternalInput").ap()
    wout_d = nc.dram_tensor("wout", [HD, DP], BF16, kind="ExternalInput").ap()
    # small fp32 vectors, packed on host:
    # cvec[:, 0]=qb_pk, 1=kb_pk, 2=gbf, 3=bb_pk, 4=bout, 5=ones
    cvec_d = nc.dram_tensor("cvec", [128, 6], F32, kind="ExternalInput").ap()
    vbrow_d = nc.dram_tensor("vbrow", [1, TC * HD], F32, kind="ExternalInput").ap()
    boutrow_d = nc.dram_tensor("boutrow", [1, DP], BF16, kind="ExternalInput").ap()
    ident_d = nc.dram_tensor("ident", [128, 128], BF16, kind="ExternalInput").ap()
    # per-core row->(chunk, partition) selector: rowsel[k, ic, m] = 1 iff the
    # core's local bias row k is global row i = 128*ic + m
    rowsel_d = nc.dram_tensor("rowsel", [128, TC, 128], BF16, kind="ExternalInput").ap()
    out_d = nc.dram_tensor("out", [NLOC, L, DP], F32, kind="ExternalOutput").ap()

    with tile.TileContext(nc) as tc, tc.tile_pool(name="persist", bufs=1) as pp:
        # ------------------------------------------------------- persistent SBUF
        wq_sb = pp.tile([DP, HD], BF16, name="wq_sb")
        wk_sb = pp.tile([DP, HD], BF16, name="wk_sb")
        wv_sb = pp.tile([DP, HD], BF16, name="wv_sb")
        wg_sb = pp.tile([DP, HD], BF16, name="wg_sb")
        wb_sb = pp.tile([DB, 32], BF16, name="wb_sb")
        wout_sb = pp.tile([HD, DP], BF16, name="wout_sb")
        cvec_sb = pp.tile([128, 6], F32, name="cvec_sb")
        vb_sb = pp.tile([128, TC * HD], F32, name="vb_sb")
        bout_sb = pp.tile([1, DP], BF16, name="bout_sb")
        ones_sb = pp.tile([1, 128], BF16, name="ones_sb")
        ident_sb = pp.tile([128, 128], BF16, name="ident_sb")
        rowsel_sb = pp.tile([128, TC, 128], BF16, name="rowsel_sb")
        bh_sb = pp.tile([128, H, L], BF16, name="bh_sb")

        nc.gpsimd.dma_start(out=wq_sb[:], in_=wq_d[:])
        nc.gpsimd.dma_start(out=wk_sb[:], in_=wk_d[:])
        nc.gpsimd.dma_start(out=wv_sb[:], in_=wv_d[:])
        nc.gpsimd.dma_start(out=wg_sb[:], in_=wg_d[:])
        nc.gpsimd.dma_start(out=wb_sb[:], in_=wb_d[:])
        nc.gpsimd.dma_start(out=wout_sb[:], in_=wout_d[:])
        nc.gpsimd.dma_start(out=cvec_sb[:], in_=cvec_d[:])
        nc.sync.dma_start(out=ident_sb[:], in_=ident_d[:])
        nc.sync.dma_start(out=rowsel_sb[:], in_=rowsel_d[:])
        nc.gpsimd.memset(bh_sb[:], 0.0)
        if has_vb:
            nc.gpsimd.dma_start(out=vb_sb[:], in_=vbrow_d.to_broadcast((128, TC * HD)))
        if has_bout:
            nc.gpsimd.dma_start(out=bout_sb[:], in_=boutrow_d[:])
            nc.vector.memset(ones_sb[:], 1.0)

        qpk_sb = pp.tile([128, 12, H, L], BF16, name="qpk_sb")   # [(ns,d), g, h, i]
        kpk_sb = pp.tile([128, 12, H, L], BF16, name="kpk_sb")
        v_tiles = [pp.tile([128, TC, HD], BF16, name=f"v{n}") for n in range(NLOC)]
        g_tiles = [pp.tile([128, L], BF16, name=f"g{n}") for n in range(NLOC)]
        a_tiles = {(h, ic): pp.tile([128, L], BF16, name=f"a{h}_{ic}")
                   for h in range(H) for ic in range(TC)}
        at_tiles = {(h, jc): pp.tile([128, L], BF16, name=f"at{h}_{jc}")
                    for h in range(H) for jc in range(TC)}
        sums_sb = pp.tile([128, H, TC], F32, name="sums_sb")
        recip_sb = pp.tile([128, H, TC], F32, name="recip_sb")

        # ------------------------------------------------------------- DRAM
        with tc.tile_pool(name="dram", bufs=1, space="DRAM") as dram:
            # bias rows round-trip: b_sb partition layout (ii, h) -> row-major
            bchunk = dram.tile([12, 128, L], BF16)
            bounce_in = dram.tile([H, L, L], BF16)
            bounce_outs = [dram.tile([2, L, L], BF16, addr_space="Shared",
                                     name=f"bout{hp}") for hp in range(2)]

            # =======================================================
            # Phases A (pair) + B (bias), interleaved waves of W rows
            # =======================================================
            with (
                tc.tile_pool(name="xt", bufs=17) as xt_pool,
                tc.tile_pool(name="xn", bufs=6) as xn_pool,
                tc.tile_pool(name="stw", bufs=2) as st_pool,
                tc.tile_pool(name="stb", bufs=18) as sb_pool,
                tc.tile_pool(name="pnt", bufs=10) as pnt_pool,
                tc.tile_pool(name="bsb", bufs=3) as bsb_pool,
                tc.tile_pool(name="pst", bufs=2, space="PSUM") as pst,
                tc.tile_pool(name="psQ", bufs=2, space="PSUM") as psQ,
                tc.tile_pool(name="psK", bufs=2, space="PSUM") as psK,
                tc.tile_pool(name="psmm", bufs=2, space="PSUM") as psmm,
            ):
                def emit_stats(xts, label):
                    """bn_stats per chunk + batched mean/rstd for a wave.
                    Returns (nm, y) tiles [128, W, 3]: nm = -mean, y = rstd."""
                    st = st_pool.tile([128, W, TC, 6], F32, name=f"st_{label}", tag="st")
                    for r in range(W):
                        for j in range(TC):
                            nc.vector.bn_stats(out=st[:, r, j], in_=xts[r][:, j])
                    s_t = sb_pool.tile([128, W, TC], F32, name="s_t", tag="sb")
                    p_t = sb_pool.tile([128, W, TC], F32, name="p_t", tag="sb")
                    q_t = sb_pool.tile([128, W, TC], F32, name="q_t", tag="sb")
                    t_t = sb_pool.tile([128, W, TC], F32, name="t_t", tag="sb")
                    ha = sb_pool.tile([128, W, TC], F32, name="ha", tag="sb")
                    y_t = sb_pool.tile([128, W, TC], F32, name="y_t", tag="sb")
                    w_t = sb_pool.tile([128, W, TC], F32, name="w_t", tag="sb")
                    nm = sb_pool.tile([128, W, TC], F32, name="nm", tag="sb")
                    me = st[:, :, :, 1]
                    mo = st[:, :, :, 4]
                    ve = st[:, :, :, 2]
                    vo = st[:, :, :, 5]
                    # var = (ve+vo)/128 + 0.25*(me+mo)^2 - me*mo ; nm = -(me+mo)/2
                    nc.vector.tensor_tensor(out=s_t[:], in0=me, in1=mo, op=ALU.add)
                    nc.gpsimd.tensor_tensor(out=p_t[:], in0=me, in1=mo, op=ALU.mult)
                    nc.gpsimd.tensor_tensor(out=q_t[:], in0=ve, in1=vo, op=ALU.add)
                    nc.vector.scalar_tensor_tensor(
                        out=t_t[:], in0=s_t[:], scalar=0.25, in1=s_t[:],
                        op0=ALU.mult, op1=ALU.mult)
                    nc.vector.scalar_tensor_tensor(
                        out=q_t[:], in0=q_t[:], scalar=1.0 / 128, in1=t_t[:],
                        op0=ALU.mult, op1=ALU.add)
                    # va = q - p + eps  (reuse t_t)
                    nc.vector.scalar_tensor_tensor(
                        out=t_t[:], in0=p_t[:], scalar=-1.0, in1=q_t[:],
                        op0=ALU.mult, op1=ALU.add)
                    nc.vector.tensor_scalar(
                        out=t_t[:], in0=t_t[:], scalar1=EPS, scalar2=None,
                        op0=ALU.add)
                    nc.vector.tensor_scalar(
                        out=ha[:], in0=t_t[:], scalar1=0.5, scalar2=None,
                        op0=ALU.mult)
                    # rsqrt: max-of-tangents seed + 3 Newton iterations
                    nc.vector.tensor_scalar(
                        out=y_t[:], in0=t_t[:], scalar1=-1.41421356,
                        scalar2=2.12132034, op0=ALU.mult, op1=ALU.add)
                    nc.vector.tensor_scalar(
                        out=w_t[:], in0=t_t[:], scalar1=-0.27216553,
                        scalar2=1.22474487, op0=ALU.mult, op1=ALU.add)
                    nc.vector.tensor_tensor(out=y_t[:], in0=y_t[:], in1=w_t[:],
                                            op=ALU.max)
                    for _ in range(2):
                        nc.vector.tensor_tensor(out=w_t[:], in0=y_t[:],
                                                in1=y_t[:], op=ALU.mult)
                        nc.vector.tensor_tensor(out=w_t[:], in0=w_t[:],
                                                in1=ha[:], op=ALU.mult)
                        nc.vector.tensor_scalar(out=w_t[:], in0=w_t[:],
                                                scalar1=-1.0, scalar2=1.5,
                                                op0=ALU.mult, op1=ALU.add)
                        nc.vector.tensor_tensor(out=y_t[:], in0=y_t[:],
                                                in1=w_t[:], op=ALU.mult)
                    nc.vector.tensor_scalar(out=nm[:], in0=s_t[:], scalar1=-0.5,
                                            scalar2=None, op0=ALU.mult)
                    nmy = sb_pool.tile([128, W, TC], F32, name="nmy", tag="sb")
                    nc.gpsimd.tensor_tensor(out=nmy[:], in0=nm[:], in1=y_t[:],
                                            op=ALU.mult)
                    return nm, y_t, nmy

                def norm_transpose(xt, nm, y, nmy, r, label):
                    """normalize (Pool) + PE transpose; returns pnT SBUF tile."""
                    xn = xn_pool.tile([128, TC, DP], BF16, name=f"xn_{label}", tag="xn")
                    psT = pst.tile([128, L], BF16, name=f"psT_{label}", tag="t")
                    for j in range(TC):
                        if j == 1:
                            nc.scalar.activation(
                                out=xn[:, j], in_=xt[:, j], func=AF.Identity,
                                bias=nmy[:, r, j:j + 1], scale=y[:, r, j:j + 1])
                        else:
                            eng = nc.vector if j == 0 else nc.gpsimd
                            eng.tensor_scalar(
                                out=xn[:, j], in0=xt[:, j],
                                scalar1=nm[:, r, j:j + 1],
                                scalar2=y[:, r, j:j + 1],
                                op0=ALU.add, op1=ALU.mult)
                        nc.tensor.transpose(
                            out=psT[:, j * 128:(j + 1) * 128],
                            in_=xn[:, j], identity=ident_sb[:])
                    pnT = pnt_pool.tile([128, L], BF16, name=f"pnT_{label}", tag="pnT")
                    nc.vector.tensor_copy(out=pnT[:], in_=psT[:])
                    return pnT

                def pair_wave(wv_):
                    n0 = wv_ * W
                    xts = []
                    for r in range(W):
                        xt = xt_pool.tile([128, TC, DP], BF16, name="xt_a", tag="xt")
                        xts.append(xt)
                        nc.sync.dma_start(
                            out=xt[:],
                            in_=pair_s[n0 + r].rearrange("(c p) d -> p c d", p=128))
                    nm, y, nmy = emit_stats(xts, "a")
                    pnts = []
                    for r in range(W):
                        pnts.append(norm_transpose(xts[r], nm, y, nmy, r, "a"))
                    for r in range(W):
                        n = n0 + r
                        pnT = pnts[r]
                        # v
                        ps_v = psmm.tile([128, TC * HD], F32, name="ps_v", tag="mm")
                        for j in range(TC):
                            nc.tensor.matmul(
                                out=ps_v[:, j * HD:(j + 1) * HD],
                                lhsT=pnT[:, j * 128:(j + 1) * 128],
                                rhs=wv_sb[:], start=True, stop=True)
                        if has_vb:
                            nc.vector.tensor_tensor(
                                out=v_tiles[n][:], in0=ps_v[:], in1=vb_sb[:],
                                op=ALU.add)
                        else:
                            nc.scalar.copy(out=v_tiles[n][:], in_=ps_v[:])
                        # g (fused sigmoid drain)
                        ps_g = psmm.tile([128, L], F32, name="ps_g", tag="mm")
                        nc.tensor.matmul(
                            out=ps_g[:], lhsT=wg_sb[:], rhs=pnT[:],
                            start=True, stop=True)
                        nc.scalar.activation(
                            out=g_tiles[n][:], in_=ps_g[:], func=AF.Sigmoid,
                            bias=cvec_sb[:, 2:3], scale=1.0)
                    # packed q/k slabs, 2 groups of 4 rows
                    for sg in range(W // 4):
                        g = (n0 // 4) + sg
                        rows = pnts[sg * 4:(sg + 1) * 4]
                        for h in range(H):
                            ps_q = psQ.tile([128, L], F32, name="ps_q")
                            ps_k = psK.tile([128, L], F32, name="ps_k")
                            for ns in range(4):
                                nc.tensor.matmul(
                                    out=ps_q[32 * ns:32 * ns + 32, :],
                                    lhsT=wq_sb[:, h * D:(h + 1) * D],
                                    rhs=rows[ns][:], start=True, stop=True,
                                    tile_position=(0, 32 * ns))
                            for ns in range(4):
                                nc.tensor.matmul(
                                    out=ps_k[32 * ns:32 * ns + 32, :],
                                    lhsT=wk_sb[:, h * D:(h + 1) * D],
                                    rhs=rows[ns][:], start=True, stop=True,
                                    tile_position=(0, 32 * ns))
                            if has_qb:
                                nc.scalar.activation(
                                    out=qpk_sb[:, g, h], in_=ps_q[:],
                                    func=AF.Copy, bias=0.0, scale=1.0)
                                nc.vector.tensor_scalar(
                                    out=qpk_sb[:, g, h], in0=qpk_sb[:, g, h],
                                    scalar1=cvec_sb[:, 0:1], scalar2=None,
                                    op0=ALU.add)
                            else:
                                nc.scalar.copy(out=qpk_sb[:, g, h], in_=ps_q[:])
                            if has_kb:
                                nc.vector.tensor_scalar(
                                    out=kpk_sb[:, g, h], in0=ps_k[:],
                                    scalar1=cvec_sb[:, 1:2], scalar2=None,
                                    op0=ALU.add)
                            else:
                                nc.vector.tensor_copy(out=kpk_sb[:, g, h],
                                                      in_=ps_k[:])

                def bias_wave(wv_):
                    i0 = wv_ * W
                    xts = []
                    for r in range(W):
                        xt = xt_pool.tile([128, TC, DB], BF16, name="xt_b", tag="xt")
                        xts.append(xt)
                        nc.sync.dma_start(
                            out=xt[:],
                            in_=bias_s[i0 + r].rearrange("(c p) d -> p c d", p=128))
                    nm, y, nmy = emit_stats(xts, "b")
                    bnTs = []
                    for r in range(W):
                        bnTs.append(norm_transpose(xts[r], nm, y, nmy, r, "b"))
                    for sg in range(W // 4):
                        k4 = (i0 // 4) + sg
                        ps_b = psmm.tile([128, L], F32, name="ps_b", tag="mm")
                        for ii in range(4):
                            nc.tensor.matmul(
                                out=ps_b[32 * ii:32 * ii + 32, :],
                                lhsT=wb_sb[:], rhs=bnTs[sg * 4 + ii][:],
                                start=True, stop=True,
                                tile_position=(0, 32 * ii))
                        b_sb = bsb_pool.tile([128, L], BF16, name="b_sb")
                        if bool(np.any(bb != 0)):
                            nc.scalar.activation(
                                out=b_sb[:], in_=ps_b[:], func=AF.Identity,
                                bias=cvec_sb[:, 3:4], scale=1.0)
                        else:
                            nc.scalar.copy(out=b_sb[:], in_=ps_b[:])
                        nc.gpsimd.dma_start(out=bchunk[k4], in_=b_sb[:])

                for wv_ in range(NW):
                    pair_wave(wv_)
                    bias_wave(wv_)

                # gather this core's bias rows [48, L] per head (zero-padded)
                for h in range(H):
                    nc.gpsimd.dma_start(
                        out=bh_sb[0:NLOC, h, :],
                        in_=bchunk[:, h:h + 97:32, :].rearrange(
                            "k i j -> (k i) j"))

            # =======================================================
            # Phase C: QK logits + bias inject, per-head AllReduce,
            # softmax; Phase D: AV, gating, output projection
            # =======================================================
            with (
                tc.tile_pool(name="psL", bufs=2, space="PSUM") as psL,
                tc.tile_pool(name="pstc", bufs=1, space="PSUM") as pstc,
                tc.tile_pool(name="psO", bufs=3, space="PSUM") as psO,
                tc.tile_pool(name="psF", bufs=2, space="PSUM") as psF,
                tc.tile_pool(name="ldr", bufs=2) as ldr_pool,
                tc.tile_pool(name="attin", bufs=3) as attin_pool,
                tc.tile_pool(name="gO", bufs=5) as gO_pool,
                tc.tile_pool(name="osb", bufs=2) as osb_pool,
            ):
                # --- C1: logits, one AllReduce per head pair
                for hp in range(2):
                    for h in (2 * hp, 2 * hp + 1):
                        for ic in range(TC):
                            ps_l = psL.tile([128, L], F32, name="ps_l")
                            for g in range(12):
                                nc.tensor.matmul(
                                    out=ps_l[:],
                                    lhsT=qpk_sb[:, g, h, ic * 128:(ic + 1) * 128],
                                    rhs=kpk_sb[:, g, h],
                                    start=(g == 0), stop=False)
                            nc.tensor.matmul(
                                out=ps_l[:], lhsT=rowsel_sb[:, ic],
                                rhs=bh_sb[:, h], start=False, stop=True)
                            ldrain = ldr_pool.tile([128, L], BF16, name="ldrain")
                            nc.vector.tensor_copy(out=ldrain[:], in_=ps_l[:])
                            nc.scalar.dma_start(
                                out=bounce_in[h, ic * 128:(ic + 1) * 128, :],
                                in_=ldrain[:])
                    nc.gpsimd.collective_compute(
                        "AllReduce",
                        mybir.AluOpType.add,
                        replica_groups=[list(range(NCORES))],
                        ins=[bounce_in[2 * hp:2 * hp + 2].opt()],
                        outs=[bounce_outs[hp][:].opt()],
                    )

                # --- C2: softmax per head (redundant on every core)
                for h in range(H):
                    for ic in range(TC):
                        att = attin_pool.tile([128, L], BF16, name="att")
                        nc.gpsimd.dma_start(
                            out=att[:],
                            in_=bounce_outs[h // 2][h % 2,
                                                    ic * 128:(ic + 1) * 128, :])
                        nc.scalar.activation(
                            out=a_tiles[(h, ic)][:], in_=att[:], func=AF.Exp,
                            bias=0.0, scale=1.0,
                            accum_out=sums_sb[:, h, ic:ic + 1])
                    nc.vector.reciprocal(out=recip_sb[:, h], in_=sums_sb[:, h])
                    for ic in range(TC):
                        nc.vector.tensor_scalar(
                            out=a_tiles[(h, ic)][:], in0=a_tiles[(h, ic)][:],
                            scalar1=recip_sb[:, h, ic:ic + 1], scalar2=None,
                            op0=ALU.mult)
                    for jc in range(TC):
                        psT = pstc.tile([128, L], BF16, name="psT_c")
                        for ic in range(TC):
                            nc.tensor.transpose(
                                out=psT[:, ic * 128:(ic + 1) * 128],
                                in_=a_tiles[(h, ic)][:, jc * 128:(jc + 1) * 128],
                                identity=ident_sb[:])
                        nc.vector.tensor_copy(out=at_tiles[(h, jc)][:], in_=psT[:])

                # --- D: AV, gating, output projection
                for nb in range(12):
                    gOs = []
                    for nn in range(4):
                        n = 4 * nb + nn
                        ps_o = psO.tile([128, L], F32, name="ps_o")
                        for jc in range(TC):
                            for h in range(H):
                                nc.tensor.matmul(
                                    out=ps_o[32 * h:32 * h + 32, :],
                                    lhsT=v_tiles[n][:, jc, h * D:(h + 1) * D],
                                    rhs=at_tiles[(h, jc)][:],
                                    start=(jc == 0),
                                    stop=(jc == TC - 1),
                                    tile_position=(0, 32 * h),
                                    skip_group_check=True)
                        gO = gO_pool.tile([128, L], BF16, name="gO")
                        gOs.append(gO)
                        nc.vector.tensor_tensor(
                            out=gO[:], in0=ps_o[:], in1=g_tiles[n][:],
                            op=ALU.mult)
                    for jt in range(TC):
                        ps_f = psF.tile([128, 4 * DP], F32, name="ps_f")
                        for nn in range(4):
                            nc.tensor.matmul(
                                out=ps_f[:, nn * DP:(nn + 1) * DP],
                                lhsT=gOs[nn][:, jt * 128:(jt + 1) * 128],
                                rhs=wout_sb[:], start=True, stop=not has_bout)
                            if has_bout:
                                nc.tensor.matmul(
                                    out=ps_f[:, nn * DP:(nn + 1) * DP],
                                    lhsT=ones_sb[:],
                                    rhs=bout_sb[:],
                                    start=False, stop=True)
                        out_sb = osb_pool.tile([128, 4 * DP], F32, name="out_sb")
                        nc.scalar.copy(out=out_sb[:], in_=ps_f[:])
                        nc.sync.dma_start(
                            out=out_d[4 * nb:4 * nb + 4,
                                      jt * 128:(jt + 1) * 128, :]
                            .rearrange("n t d -> t n d"),
                            in_=out_sb.rearrange("t (n d) -> t n d", n=4))

    return nc


def prepare(pair, bias, gamma_p, beta_p, gamma_b, beta_b,
            Wq, Wk, Wv, Wb, Wg, bg, Wout, bout):
    """Fold weights, build the program, shard inputs. Returns (nc, in_maps)."""
    pair = np.asarray(pair, np.float32)
    bias = np.asarray(bias, np.float32)
    gamma_p = np.asarray(gamma_p, np.float32)
    beta_p = np.asarray(beta_p, np.float32)
    gamma_b = np.asarray(gamma_b, np.float32)
    beta_b = np.asarray(beta_b, np.float32)
    Wq = np.asarray(Wq, np.float32)
    Wk = np.asarray(Wk, np.float32)
    Wv = np.asarray(Wv, np.float32)
    Wb = np.asarray(Wb, np.float32)
    Wg = np.asarray(Wg, np.float32)
    bg = np.asarray(bg, np.float32)
    Wout = np.asarray(Wout, np.float32)
    bout = np.asarray(bout, np.float32)

    scaling = 1.0 / math.sqrt(D)
    wq = gamma_p[:, None] * Wq * scaling
    wk = gamma_p[:, None] * Wk / L
    wv = gamma_p[:, None] * Wv
    wg = gamma_p[:, None] * Wg
    wb = gamma_b[:, None] * Wb
    qb = beta_p @ Wq * scaling
    kb = beta_p @ Wk / L
    vb = beta_p @ Wv
    gbf = beta_p @ Wg + bg
    bb = beta_b @ Wb
    # packed per-partition bias columns
    bb_pk = np.zeros(128, np.float32)
    cvec = np.zeros((128, 6), np.float32)
    cvec[:, 2] = gbf
    cvec[:, 4] = bout
    cvec[:, 5] = 1.0
    for k4 in range(4):
        for h in range(H):
            bb_pk[32 * k4 + h] = bb[h]
    cvec[:, 3] = bb_pk
    has_qb = bool(np.any(qb != 0))
    has_kb = bool(np.any(kb != 0))
    if has_qb or has_kb:
        qh = qb.reshape(H, D)
        kh = kb.reshape(H, D)
        if not (np.allclose(qh, qh[0:1]) and np.allclose(kh, kh[0:1])):
            raise NotImplementedError("head-dependent q/k bias not supported")
        cvec[:, 0] = np.tile(qh[0], 4)
        cvec[:, 1] = np.tile(kh[0], 4)
    vbrow = np.tile(vb, TC)[None, :]
    wbp = np.zeros((DB, 32), np.float32)
    wbp[:, :H] = wb

    nc = build_program(wq, wk, wv, wg, wb, Wout, qb, kb, vb, gbf, bb, bout)

    # ------------------------------------------------------------- shard
    pair_t = np.ascontiguousarray(pair[0].transpose(1, 0, 2))  # [n, t, c]
    bias_t = np.ascontiguousarray(bias[0].transpose(1, 0, 2))  # [i, j, c]
    in_maps = []
    for c in range(NCORES):
        rowsel = np.zeros((128, TC, 128), np.float32)
        for k in range(NLOC):
            i = c * NLOC + k
            rowsel[k, i // 128, i % 128] = 1.0
        in_maps.append({
            "pair_s": pair_t[c * NLOC:(c + 1) * NLOC].astype(NPBF16),
            "bias_s": bias_t[c * NLOC:(c + 1) * NLOC].astype(NPBF16),
            "wq": wq.astype(NPBF16),
            "wk": wk.astype(NPBF16),
            "wv": wv.astype(NPBF16),
            "wg": wg.astype(NPBF16),
            "wb": wbp.astype(NPBF16),
            "wout": Wout.astype(NPBF16),
            "cvec": cvec,
            "vbrow": vbrow,
            "boutrow": bout[None, :].astype(NPBF16),
            "ident": np.eye(128, dtype=np.float32).astype(NPBF16),
            "rowsel": rowsel.astype(NPBF16),
        })
    return nc, in_maps


def assemble(outs):
    """outs: list of 8 per-core [48, 384, 128] arrays -> full [1, L, L, DP]."""
    full = np.concatenate(outs, axis=0)        # [384 n, 384 i, 128]
    final = full.transpose(1, 0, 2)[None]      # [1, i, n, dp] == [1, L, L, DP]
    return np.ascontiguousarray(final, dtype=np.float32)


def kernel(pair, bias, gamma_p, beta_p, gamma_b, beta_b,
           Wq, Wk, Wv, Wb, Wg, bg, Wout, bout):
    nc, in_maps = prepare(pair, bias, gamma_p, beta_p, gamma_b, beta_b,
                          Wq, Wk, Wv, Wb, Wg, bg, Wout, bout)
    if not nc.is_finalized():
        nc.finalize()
    res = run_bass_kernel_spmd(nc, in_maps, list(range(NCORES)))
    global LAST_RESULT
    LAST_RESULT = res
    outs = [res.results[c]["out"] for c in range(NCORES)]  # [48, 384, 128] each
    return assemble(outs)


# revision 18
# speedup vs baseline: 1.0529x; 1.0161x over previous
"""Biased axial attention on 8 TRN2 NeuronCores (Bass/Tile SPMD kernel).

Sharding: leading (non-attended) L axis n across 8 cores (sequence parallel).
Each core computes partial QK logits over its 48 n-rows, adds the bias term
for the 48 i-rows it owns (selected into the right logits partitions via a
per-core selection matmul -- no AllGather needed), then one bf16 AllReduce
per head of the [i,j] logits, softmax redundantly per core, AV/gating/output
local in n.

Engine split: PE matmuls/transposes, DVE stats+copies+gating, Pool (gpsimd)
normalize + small stat math, Act q/v drains + fused sigmoid g drain + exp.
rstd is computed on DVE with a tangent-seed + 3 Newton iterations (no scalar
Sqrt -> no activation-table thrash; Act only ever loads sigmoid + exp sets).
"""

import math
import numpy as np
import ml_dtypes

import concourse.bass as bass
import concourse.bacc as bacc
import concourse.tile as tile
from concourse import mybir
from concourse.bass_utils import run_bass_kernel_spmd

BF16 = mybir.dt.bfloat16
F32 = mybir.dt.float32
NPBF16 = ml_dtypes.bfloat16

LAST_RESULT = None  # BassKernelResults of the most recent run (for test.py)

NCORES = 8
L = 384
DP = 128  # pair channels
DB = 128  # bias channels
H = 4
D = 32
HD = H * D  # 128
NLOC = L // NCORES  # 48 rows per core
TC = 3  # token chunks of 128
EPS = 1e-5
W = 8  # rows per wave
NW = NLOC // W  # 6 waves per path
AF = mybir.ActivationFunctionType
ALU = mybir.AluOpType


def build_program(wq, wk, wv, wg, wb, wout, qb, kb, vb, gbf, bb, bout):
    """Build the SPMD Bass program. Weight args are numpy f32 host copies used
    only to decide which (usually-zero) bias paths to emit."""
    has_qb = bool(np.any(qb != 0))
    has_kb = bool(np.any(kb != 0))
    has_vb = bool(np.any(vb != 0))
    has_bout = bool(np.any(bout != 0))

    nc = bacc.Bacc(
        "TRN2",
        target_bir_lowering=False,
        debug=False,
        enable_asserts=False,
        num_devices=NCORES,
    )

    # ------------------------------------------------------------------ I/O
    pair_s = nc.dram_tensor("pair_s", [NLOC, L, DP], BF16, kind="ExternalInput").ap()
    bias_s = nc.dram_tensor("bias_s", [NLOC, L, DB], BF16, kind="ExternalInput").ap()
    wq_d = nc.dram_tensor("wq", [DP, HD], BF16, kind="ExternalInput").ap()
    wk_d = nc.dram_tensor("wk", [DP, HD], BF16, kind="ExternalInput").ap()
    wv_d = nc.dram_tensor("wv", [DP, HD], BF16, kind="ExternalInput").ap()
    wg_d = nc.dram_tensor("wg", [DP, HD], BF16, kind="ExternalInput").ap()
    wb_d = nc.dram_tensor("wb", [DB, 32], BF16, kind="ExternalInput").ap()
    wout_d = nc.dram_tensor("wout", [HD, DP], BF16, kind="ExternalInput").ap()
    # small fp32 vectors, packed on host:
    # cvec[:, 0]=qb_pk, 1=kb_pk, 2=gbf, 3=bb_pk, 4=bout, 5=ones
    cvec_d = nc.dram_tensor("cvec", [128, 6], F32, kind="ExternalInput").ap()
    vbrow_d = nc.dram_tensor("vbrow", [1, TC * HD], F32, kind="ExternalInput").ap()
    boutrow_d = nc.dram_tensor("boutrow", [1, DP], BF16, kind="ExternalInput").ap()
    ident_d = nc.dram_tensor("ident", [128, 128], BF16, kind="ExternalInput").ap()
    # per-core row->(chunk, partition) selector: rowsel[k, ic, m] = 1 iff the
    # core's local bias row k is global row i = 128*ic + m
    rowsel_d = nc.dram_tensor("rowsel", [128, TC, 128], BF16, kind="ExternalInput").ap()
    out_d = nc.dram_tensor("out", [NLOC, L, DP], F32, kind="ExternalOutput").ap()

    with tile.TileContext(nc) as tc, tc.tile_pool(name="persist", bufs=1) as pp:
        # ------------------------------------------------------- persistent SBUF
        wq_sb = pp.tile([DP, HD], BF16, name="wq_sb")
        wk_sb = pp.tile([DP, HD], BF16, name="wk_sb")
        wv_sb = pp.tile([DP, HD], BF16, name="wv_sb")
        wg_sb = pp.tile([DP, HD], BF16, name="wg_sb")
        wb_sb = pp.tile([DB, 32], BF16, name="wb_sb")
        wout_sb = pp.tile([HD, DP], BF16, name="wout_sb")
        cvec_sb = pp.tile([128, 6], F32, name="cvec_sb")
        vb_sb = pp.tile([128, TC * HD], F32, name="vb_sb")
        bout_sb = pp.tile([1, DP], BF16, name="bout_sb")
        ones_sb = pp.tile([1, 128], BF16, name="ones_sb")
        ident_sb = pp.tile([128, 128], BF16, name="ident_sb")
        rowsel_sb = pp.tile([128, TC, 128], BF16, name="rowsel_sb")
        bh_sb = pp.tile([128, H, L], BF16, name="bh_sb")

        nc.gpsimd.dma_start(out=wq_sb[:], in_=wq_d[:])
        nc.gpsimd.dma_start(out=wk_sb[:], in_=wk_d[:])
        nc.gpsimd.dma_start(out=wv_sb[:], in_=wv_d[:])
        nc.gpsimd.dma_start(out=wg_sb[:], in_=wg_d[:])
        nc.gpsimd.dma_start(out=wb_sb[:], in_=wb_d[:])
        nc.gpsimd.dma_start(out=wout_sb[:], in_=wout_d[:])
        nc.gpsimd.dma_start(out=cvec_sb[:], in_=cvec_d[:])
        nc.sync.dma_start(out=ident_sb[:], in_=ident_d[:])
        nc.sync.dma_start(out=rowsel_sb[:], in_=rowsel_d[:])
        nc.gpsimd.memset(bh_sb[:], 0.0)
        if has_vb:
            nc.gpsimd.dma_start(out=vb_sb[:], in_=vbrow_d.to_broadcast((128, TC * HD)))
        if has_bout:
            nc.gpsimd.dma_start(out=bout_sb[:], in_=boutrow_d[:])
            nc.vector.memset(ones_sb[:], 1.0)

        qpk_sb = pp.tile([128, 12, H, L], BF16, name="qpk_sb")   # [(ns,d), g, h, i]
        kpk_sb = pp.tile([128, 12, H, L], BF16, name="kpk_sb")
        v_tiles = [pp.tile([128, TC, HD], BF16, name=f"v{n}") for n in range(NLOC)]
        g_tiles = [pp.tile([128, L], BF16, name=f"g{n}") for n in range(NLOC)]
        a_tiles = {(h, ic): pp.tile([128, L], BF16, name=f"a{h}_{ic}")
                   for h in range(H) for ic in range(TC)}
        at_tiles = {(h, jc): pp.tile([128, L], BF16, name=f"at{h}_{jc}")
                    for h in range(H) for jc in range(TC)}
        sums_sb = pp.tile([128, H, TC], F32, name="sums_sb")
        recip_sb = pp.tile([128, H, TC], F32, name="recip_sb")

        # ------------------------------------------------------------- DRAM
        with tc.tile_pool(name="dram", bufs=1, space="DRAM") as dram:
            # bias rows round-trip: b_sb partition layout (ii, h) -> row-major
            bchunk = dram.tile([12, 128, L], BF16)
            bounce_in = dram.tile([H, L, L], BF16)
            bounce_outs = [dram.tile([2, L, L], BF16, addr_space="Shared",
                                     name=f"bout{hp}") for hp in range(2)]

            # =======================================================
            # Phases A (pair) + B (bias), interleaved waves of W rows
            # =======================================================
            with (
                tc.tile_pool(name="xt", bufs=17) as xt_pool,
                tc.tile_pool(name="xn", bufs=6) as xn_pool,
                tc.tile_pool(name="stw", bufs=2) as st_pool,
                tc.tile_pool(name="stb", bufs=18) as sb_pool,
                tc.tile_pool(name="pnt", bufs=10) as pnt_pool,
                tc.tile_pool(name="bsb", bufs=3) as bsb_pool,
                tc.tile_pool(name="pst", bufs=2, space="PSUM") as pst,
                tc.tile_pool(name="psQ", bufs=2, space="PSUM") as psQ,
                tc.tile_pool(name="psK", bufs=2, space="PSUM") as psK,
                tc.tile_pool(name="psmm", bufs=2, space="PSUM") as psmm,
            ):
                def emit_stats(xts, label):
                    """bn_stats per chunk + batched mean/rstd for a wave.
                    Returns (nm, y) tiles [128, W, 3]: nm = -mean, y = rstd."""
                    st = st_pool.tile([128, W, TC, 6], F32, name=f"st_{label}", tag="st")
                    for r in range(W):
                        for j in range(TC):
                            nc.vector.bn_stats(out=st[:, r, j], in_=xts[r][:, j])
                    s_t = sb_pool.tile([128, W, TC], F32, name="s_t", tag="sb")
                    p_t = sb_pool.tile([128, W, TC], F32, name="p_t", tag="sb")
                    q_t = sb_pool.tile([128, W, TC], F32, name="q_t", tag="sb")
                    t_t = sb_pool.tile([128, W, TC], F32, name="t_t", tag="sb")
                    ha = sb_pool.tile([128, W, TC], F32, name="ha", tag="sb")
                    y_t = sb_pool.tile([128, W, TC], F32, name="y_t", tag="sb")
                    w_t = sb_pool.tile([128, W, TC], F32, name="w_t", tag="sb")
                    nm = sb_pool.tile([128, W, TC], F32, name="nm", tag="sb")
                    me = st[:, :, :, 1]
                    mo = st[:, :, :, 4]
                    ve = st[:, :, :, 2]
                    vo = st[:, :, :, 5]
                    # var = (ve+vo)/128 + 0.25*(me+mo)^2 - me*mo ; nm = -(me+mo)/2
                    nc.vector.tensor_tensor(out=s_t[:], in0=me, in1=mo, op=ALU.add)
                    nc.gpsimd.tensor_tensor(out=p_t[:], in0=me, in1=mo, op=ALU.mult)
                    nc.gpsimd.tensor_tensor(out=q_t[:], in0=ve, in1=vo, op=ALU.add)
                    nc.vector.scalar_tensor_tensor(
                        out=t_t[:], in0=s_t[:], scalar=0.25, in1=s_t[:],
                        op0=ALU.mult, op1=ALU.mult)
                    nc.vector.scalar_tensor_tensor(
                        out=q_t[:], in0=q_t[:], scalar=1.0 / 128, in1=t_t[:],
                        op0=ALU.mult, op1=ALU.add)
                    # va = q - p + eps  (reuse t_t)
                    nc.vector.scalar_tensor_tensor(
                        out=t_t[:], in0=p_t[:], scalar=-1.0, in1=q_t[:],
                        op0=ALU.mult, op1=ALU.add)
                    nc.vector.tensor_scalar(
                        out=t_t[:], in0=t_t[:], scalar1=EPS, scalar2=None,
                        op0=ALU.add)
                    nc.vector.tensor_scalar(
                        out=ha[:], in0=t_t[:], scalar1=0.5, scalar2=None,
                        op0=ALU.mult)
                    # rsqrt: max-of-tangents seed + 3 Newton iterations
                    nc.vector.tensor_scalar(
                        out=y_t[:], in0=t_t[:], scalar1=-1.41421356,
                        scalar2=2.12132034, op0=ALU.mult, op1=ALU.add)
                    nc.vector.tensor_scalar(
                        out=w_t[:], in0=t_t[:], scalar1=-0.27216553,
                        scalar2=1.22474487, op0=ALU.mult, op1=ALU.add)
                    nc.vector.tensor_tensor(out=y_t[:], in0=y_t[:], in1=w_t[:],
                                            op=ALU.max)
                    for _ in range(2):
                        nc.vector.tensor_tensor(out=w_t[:], in0=y_t[:],
                                                in1=y_t[:], op=ALU.mult)
                        nc.vector.tensor_tensor(out=w_t[:], in0=w_t[:],
                                                in1=ha[:], op=ALU.mult)
                        nc.vector.tensor_scalar(out=w_t[:], in0=w_t[:],
                                                scalar1=-1.0, scalar2=1.5,
                                                op0=ALU.mult, op1=ALU.add)
                        nc.vector.tensor_tensor(out=y_t[:], in0=y_t[:],
                                                in1=w_t[:], op=ALU.mult)
                    nc.vector.tensor_scalar(out=nm[:], in0=s_t[:], scalar1=-0.5,
                                            scalar2=None, op0=ALU.mult)
                    nmy = sb_pool.tile([128, W, TC], F32, name="nmy", tag="sb")
                    nc.gpsimd.tensor_tensor(out=nmy[:], in0=nm[:], in1=y_t[:],
                                            op=ALU.mult)
                    return nm, y_t, nmy

                def norm_transpose(xt, nm, y, nmy, r, label):
                    """normalize (Pool) + PE transpose; returns pnT SBUF tile."""
                    xn = xn_pool.tile([128, TC, DP], BF16, name=f"xn_{label}", tag="xn")
                    psT = pst.tile([128, L], BF16, name=f"psT_{label}", tag="t")
                    for j in range(TC):
                        if j == 1:
                            nc.scalar.activation(
                                out=xn[:, j], in_=xt[:, j], func=AF.Identity,
                                bias=nmy[:, r, j:j + 1], scale=y[:, r, j:j + 1])
                        else:
                            eng = nc.vector if j == 0 else nc.gpsimd
                            eng.tensor_scalar(
                                out=xn[:, j], in0=xt[:, j],
                                scalar1=nm[:, r, j:j + 1],
                                scalar2=y[:, r, j:j + 1],
                                op0=ALU.add, op1=ALU.mult)
                        nc.tensor.transpose(
                            out=psT[:, j * 128:(j + 1) * 128],
                            in_=xn[:, j], identity=ident_sb[:])
                    pnT = pnt_pool.tile([128, L], BF16, name=f"pnT_{label}", tag="pnT")
                    nc.vector.tensor_copy(out=pnT[:], in_=psT[:])
                    return pnT

                def pair_wave(wv_):
                    n0 = wv_ * W
                    xts = []
                    for r in range(W):
                        xt = xt_pool.tile([128, TC, DP], BF16, name="xt_a", tag="xt")
                        xts.append(xt)
                        nc.sync.dma_start(
                            out=xt[:],
                            in_=pair_s[n0 + r].rearrange("(c p) d -> p c d", p=128))
                    nm, y, nmy = emit_stats(xts, "a")
                    pnts = []
                    for r in range(W):
                        pnts.append(norm_transpose(xts[r], nm, y, nmy, r, "a"))
                    for r in range(W):
                        n = n0 + r
                        pnT = pnts[r]
                        # v
                        ps_v = psmm.tile([128, TC * HD], F32, name="ps_v", tag="mm")
                        for j in range(TC):
                            nc.tensor.matmul(
                                out=ps_v[:, j * HD:(j + 1) * HD],
                                lhsT=pnT[:, j * 128:(j + 1) * 128],
                                rhs=wv_sb[:], start=True, stop=True)
                        if has_vb:
                            nc.vector.tensor_tensor(
                                out=v_tiles[n][:], in0=ps_v[:], in1=vb_sb[:],
                                op=ALU.add)
                        else:
                            nc.scalar.copy(out=v_tiles[n][:], in_=ps_v[:])
                        # g (fused sigmoid drain)
                        ps_g = psmm.tile([128, L], F32, name="ps_g", tag="mm")
                        nc.tensor.matmul(
                            out=ps_g[:], lhsT=wg_sb[:], rhs=pnT[:],
                            start=True, stop=True)
                        nc.scalar.activation(
                            out=g_tiles[n][:], in_=ps_g[:], func=AF.Sigmoid,
                            bias=cvec_sb[:, 2:3], scale=1.0)
                    # packed q/k slabs, 2 groups of 4 rows
                    for sg in range(W // 4):
                        g = (n0 // 4) + sg
                        rows = pnts[sg * 4:(sg + 1) * 4]
                        for h in range(H):
                            ps_q = psQ.tile([128, L], F32, name="ps_q")
                            ps_k = psK.tile([128, L], F32, name="ps_k")
                            for ns in range(4):
                                nc.tensor.matmul(
                                    out=ps_q[32 * ns:32 * ns + 32, :],
                                    lhsT=wq_sb[:, h * D:(h + 1) * D],
                                    rhs=rows[ns][:], start=True, stop=True,
                                    tile_position=(0, 32 * ns))
                            for ns in range(4):
                                nc.tensor.matmul(
                                    out=ps_k[32 * ns:32 * ns + 32, :],
                                    lhsT=wk_sb[:, h * D:(h + 1) * D],
                                    rhs=rows[ns][:], start=True, stop=True,
                                    tile_position=(0, 32 * ns))
                            if has_qb:
                                nc.scalar.activation(
                                    out=qpk_sb[:, g, h], in_=ps_q[:],
                                    func=AF.Copy, bias=0.0, scale=1.0)
                                nc.vector.tensor_scalar(
                                    out=qpk_sb[:, g, h], in0=qpk_sb[:, g, h],
                                    scalar1=cvec_sb[:, 0:1], scalar2=None,
                                    op0=ALU.add)
                            else:
                                nc.scalar.copy(out=qpk_sb[:, g, h], in_=ps_q[:])
                            if has_kb:
                                nc.vector.tensor_scalar(
                                    out=kpk_sb[:, g, h], in0=ps_k[:],
                                    scalar1=cvec_sb[:, 1:2], scalar2=None,
                                    op0=ALU.add)
                            else:
                                nc.vector.tensor_copy(out=kpk_sb[:, g, h],
                                                      in_=ps_k[:])

                def bias_wave(wv_):
                    i0 = wv_ * W
                    xts = []
                    for r in range(W):
                        xt = xt_pool.tile([128, TC, DB], BF16, name="xt_b", tag="xt")
                        xts.append(xt)
                        nc.sync.dma_start(
                            out=xt[:],
                            in_=bias_s[i0 + r].rearrange("(c p) d -> p c d", p=128))
                    nm, y, nmy = emit_stats(xts, "b")
                    bnTs = []
                    for r in range(W):
                        bnTs.append(norm_transpose(xts[r], nm, y, nmy, r, "b"))
                    for sg in range(W // 4):
                        k4 = (i0 // 4) + sg
                        ps_b = psmm.tile([128, L], F32, name="ps_b", tag="mm")
                        for ii in range(4):
                            nc.tensor.matmul(
                                out=ps_b[32 * ii:32 * ii + 32, :],
                                lhsT=wb_sb[:], rhs=bnTs[sg * 4 + ii][:],
                                start=True, stop=True,
                                tile_position=(0, 32 * ii))
                        b_sb = bsb_pool.tile([128, L], BF16, name="b_sb")
                        if bool(np.any(bb != 0)):
                            nc.scalar.activation(
                                out=b_sb[:], in_=ps_b[:], func=AF.Identity,
                                bias=cvec_sb[:, 3:4], scale=1.0)
                        else:
                            nc.scalar.copy(out=b_sb[:], in_=ps_b[:])
                        nc.gpsimd.dma_start(out=bchunk[k4], in_=b_sb[:])

                for wv_ in range(NW):
                    pair_wave(wv_)
                    bias_wave(wv_)

                # gather this core's bias rows [48, L] per head (zero-padded)
                for h in range(H):
                    nc.gpsimd.dma_start(
                        out=bh_sb[0:NLOC, h, :],
                        in_=bchunk[:, h:h + 97:32, :].rearrange(
                            "k i j -> (k i) j"))

            # =======================================================
            # Phase C: QK logits + bias inject, per-head AllReduce,
            # softmax; Phase D: AV, gating, output projection
            # =======================================================
            with (
                tc.tile_pool(name="psL", bufs=2, space="PSUM") as psL,
                tc.tile_pool(name="pstc", bufs=1, space="PSUM") as pstc,
                tc.tile_pool(name="psO", bufs=3, space="PSUM") as psO,
                tc.tile_pool(name="psF", bufs=2, space="PSUM") as psF,
                tc.tile_pool(name="ldr", bufs=2) as ldr_pool,
                tc.tile_pool(name="attin", bufs=3) as attin_pool,
                tc.tile_pool(name="gO", bufs=5) as gO_pool,
                tc.tile_pool(name="osb", bufs=2) as osb_pool,
            ):
                # --- C1: logits, one AllReduce per head pair
                for hp in range(2):
                    for h in (2 * hp, 2 * hp + 1):
                        for ic in range(TC):
                            ps_l = psL.tile([128, L], F32, name="ps_l")
                            for g in range(12):
                                nc.tensor.matmul(
                                    out=ps_l[:],
                                    lhsT=qpk_sb[:, g, h, ic * 128:(ic + 1) * 128],
                                    rhs=kpk_sb[:, g, h],
                                    start=(g == 0), stop=False)
                            nc.tensor.matmul(
                                out=ps_l[:], lhsT=rowsel_sb[:, ic],
                                rhs=bh_sb[:, h], start=False, stop=True)
                            ldrain = ldr_pool.tile([128, L], BF16, name="ldrain")
                            nc.vector.tensor_copy(out=ldrain[:], in_=ps_l[:])
                            nc.scalar.dma_start(
                                out=bounce_in[h, ic * 128:(ic + 1) * 128, :],
                                in_=ldrain[:])
                    nc.gpsimd.collective_compute(
                        "AllReduce",
                        mybir.AluOpType.add,
                        replica_groups=[list(range(NCORES))],
                        ins=[bounce_in[2 * hp:2 * hp + 2].opt()],
                        outs=[bounce_outs[hp][:].opt()],
                    )

                # --- C2: softmax per head (redundant on every core)
                for h in range(H):
                    for ic in range(TC):
                        att = attin_pool.tile([128, L], BF16, name="att")
                        nc.gpsimd.dma_start(
                            out=att[:],
                            in_=bounce_outs[h // 2][h % 2,
                                                    ic * 128:(ic + 1) * 128, :])
                        nc.scalar.activation(
                            out=a_tiles[(h, ic)][:], in_=att[:], func=AF.Exp,
                            bias=0.0, scale=1.0,
                            accum_out=sums_sb[:, h, ic:ic + 1])
                    nc.vector.reciprocal(out=recip_sb[:, h], in_=sums_sb[:, h])
                    for ic in range(TC):
                        nc.vector.tensor_scalar(
                            out=a_tiles[(h, ic)][:], in0=a_tiles[(h, ic)][:],
                            scalar1=recip_sb[:, h, ic:ic + 1], scalar2=None,
                            op0=ALU.mult)
                    for jc in range(TC):
                        psT = pstc.tile([128, L], BF16, name="psT_c")
                        for ic in range(TC):
                            nc.tensor.transpose(
                                out=psT[:, ic * 128:(ic + 1) * 128],
                                in_=a_tiles[(h, ic)][:, jc * 128:(jc + 1) * 128],
                                identity=ident_sb[:])
                        nc.vector.tensor_copy(out=at_tiles[(h, jc)][:], in_=psT[:])

                # --- D: AV, gating, output projection
                for nb in range(12):
                    gOs = []
                    for nn in range(4):
                        n = 4 * nb + nn
                        ps_o = psO.tile([128, L], F32, name="ps_o")
                        for jc in range(TC):
                            for h in range(H):
                                nc.tensor.matmul(
                                    out=ps_o[32 * h:32 * h + 32, :],
                                    lhsT=v_tiles[n][:, jc, h * D:(h + 1) * D],
                                    rhs=at_tiles[(h, jc)][:],
                                    start=(jc == 0),
                                    stop=(jc == TC - 1),
                                    tile_position=(0, 32 * h),
                                    skip_group_check=True)
                        gO = gO_pool.tile([128, L], BF16, name="gO")
                        gOs.append(gO)
                        nc.vector.tensor_tensor(
                            out=gO[:], in0=ps_o[:], in1=g_tiles[n][:],
                            op=ALU.mult)
                    for jt in range(TC):
                        ps_f = psF.tile([128, 4 * DP], F32, name="ps_f")
                        for nn in range(4):
                            nc.tensor.matmul(
                                out=ps_f[:, nn * DP:(nn + 1) * DP],
                                lhsT=gOs[nn][:, jt * 128:(jt + 1) * 128],
                                rhs=wout_sb[:], start=True, stop=not has_bout)
                            if has_bout:
                                nc.tensor.matmul(
                                    out=ps_f[:, nn * DP:(nn + 1) * DP],
                                    lhsT=ones_sb[:],
                                    rhs=bout_sb[:],
                                    start=False, stop=True)
                        out_sb = osb_pool.tile([128, 4 * DP], F32, name="out_sb")
                        nc.scalar.copy(out=out_sb[:], in_=ps_f[:])
                        nc.sync.dma_start(
                            out=out_d[4 * nb:4 * nb + 4,
                                      jt * 128:(jt + 1) * 128, :]
                            .rearrange("n t d -> t n d"),
                            in_=out_sb.rearrange("t (n d) -> t n d", n=4))

    return nc


def prepare(pair, bias, gamma_p, beta_p, gamma_b, beta_b,
            Wq, Wk, Wv, Wb, Wg, bg, Wout, bout):
    """Fold weights, build the program, shard inputs. Returns (nc, in_maps)."""
    pair = np.asarray(pair, np.float32)
    bias = np.asarray(bias, np.float32)
    gamma_p = np.asarray(gamma_p, np.float32)
    beta_p = np.asarray(beta_p, np.float32)
    gamma_b = np.asarray(gamma_b, np.float32)
    beta_b = np.asarray(beta_b, np.float32)
    Wq = np.asarray(Wq, np.float32)
    Wk = np.asarray(Wk, np.float32)
    Wv = np.asarray(Wv, np.float32)
    Wb = np.asarray(Wb, np.float32)
    Wg = np.asarray(Wg, np.float32)
    bg = np.asarray(bg, np.float32)
    Wout = np.asarray(Wout, np.float32)
    bout = np.asarray(bout, np.float32)

    scaling = 1.0 / math.sqrt(D)
    wq = gamma_p[:, None] * Wq * scaling
    wk = gamma_p[:, None] * Wk / L
    wv = gamma_p[:, None] * Wv
    wg = gamma_p[:, None] * Wg
    wb = gamma_b[:, None] * Wb
    qb = beta_p @ Wq * scaling
    kb = beta_p @ Wk / L
    vb = beta_p @ Wv
    gbf = beta_p @ Wg + bg
    bb = beta_b @ Wb
    # packed per-partition bias columns
    bb_pk = np.zeros(128, np.float32)
    cvec = np.zeros((128, 6), np.float32)
    cvec[:, 2] = gbf
    cvec[:, 4] = bout
    cvec[:, 5] = 1.0
    for k4 in range(4):
        for h in range(H):
            bb_pk[32 * k4 + h] = bb[h]
    cvec[:, 3] = bb_pk
    has_qb = bool(np.any(qb != 0))
    has_kb = bool(np.any(kb != 0))
    if has_qb or has_kb:
        qh = qb.reshape(H, D)
        kh = kb.reshape(H, D)
        if not (np.allclose(qh, qh[0:1]) and np.allclose(kh, kh[0:1])):
            raise NotImplementedError("head-dependent q/k bias not supported")
        cvec[:, 0] = np.tile(qh[0], 4)
        cvec[:, 1] = np.tile(kh[0], 4)
    vbrow = np.tile(vb, TC)[None, :]
    wbp = np.zeros((DB, 32), np.float32)
    wbp[:, :H] = wb

    nc = build_program(wq, wk, wv, wg, wb, Wout, qb, kb, vb, gbf, bb, bout)

    # ------------------------------------------------------------- shard
    pair_t = np.ascontiguousarray(pair[0].transpose(1, 0, 2))  # [n, t, c]
    bias_t = np.ascontiguousarray(bias[0].transpose(1, 0, 2))  # [i, j, c]
    in_maps = []
    for c in range(NCORES):
        rowsel = np.zeros((128, TC, 128), np.float32)
        for k in range(NLOC):
            i = c * NLOC + k
            rowsel[k, i // 128, i % 128] = 1.0
        in_maps.append({
            "pair_s": pair_t[c * NLOC:(c + 1) * NLOC].astype(NPBF16),
            "bias_s": bias_t[c * NLOC:(c + 1) * NLOC].astype(NPBF16),
            "wq": wq.astype(NPBF16),
            "wk": wk.astype(NPBF16),
            "wv": wv.astype(NPBF16),
            "wg": wg.astype(NPBF16),
            "wb": wbp.astype(NPBF16),
            "wout": Wout.astype(NPBF16),
            "cvec": cvec,
            "vbrow": vbrow,
            "boutrow": bout[None, :].astype(NPBF16),
            "ident": np.eye(128, dtype=np.float32).astype(NPBF16),
            "rowsel": rowsel.astype(NPBF16),
        })
    return nc, in_maps


def assemble(outs):
    """outs: list of 8 per-core [48, 384, 128] arrays -> full [1, L, L, DP]."""
    full = np.concatenate(outs, axis=0)        # [384 n, 384 i, 128]
    final = full.transpose(1, 0, 2)[None]      # [1, i, n, dp] == [1, L, L, DP]
    return np.ascontiguousarray(final, dtype=np.float32)


def kernel(pair, bias, gamma_p, beta_p, gamma_b, beta_b,
           Wq, Wk, Wv, Wb, Wg, bg, Wout, bout):
    nc, in_maps = prepare(pair, bias, gamma_p, beta_p, gamma_b, beta_b,
                          Wq, Wk, Wv, Wb, Wg, bg, Wout, bout)
    if not nc.is_finalized():
        nc.finalize()
    res = run_bass_kernel_spmd(nc, in_maps, list(range(NCORES)))
    global LAST_RESULT
    LAST_RESULT = res
    outs = [res.results[c]["out"] for c in range(NCORES)]  # [48, 384, 128] each
    return assemble(outs)
